# revision 1
# baseline (speedup 1.0000x reference)
"""Multi-head attention (B=4, S=2048, C=1024, H=16) on 8 TRN2 NeuronCores.

Sharding: data-parallel over batch (4) x query-row split (2); core c handles
batch c//2, query rows [(c%2)*1024, +1024). The host rolls each core's x by
its query-row offset (attention is permutation-invariant over keys), passes
x^T and DMA-friendly re-layouts of the weights, and each core runs:

  A) QKV projection in float32r (full PE rate at N=512), Q^T/K^T produced
     feature-major, V natural; spilled to DRAM scratch.
  B) Per head-pair attention: transposed scores sc[j,i] = K_h^T(stationary)
     x Q_h^T(moving), both heads row-packed on PE array halves into one PSUM
     tile; one exp per 3-j-tile block (scale=1/8 folded into the ACT affine);
     P.V with a [V|1|0*63] 128-column stationary so the fp32r fast-weight-load
     stays on and the softmax denominator accumulates at out row 64; PV of
     block i-1 is emitted after exp of block i so it fills the PE while ACT
     runs. Normalization multiplies by the broadcast reciprocal denominator
     and folds the V bias in afterwards (sum_j softmax = 1).
  C) Out-projection with O^T as stationary so y lands in natural [row,
     channel] layout; bias added via a partition-broadcast tile.

No collectives; each core writes its own [1024, 1024] output slice.
"""

from contextlib import ExitStack

import numpy as np

import concourse.mybir as mybir
import concourse.tile as tile
from concourse import bacc
from concourse.bass_utils import run_bass_kernel_spmd
from concourse.masks import make_identity

F32 = mybir.dt.float32
F32R = mybir.dt.float32r
AF = mybir.ActivationFunctionType

B, S, C, H, DH = 4, 2048, 1024, 16, 64
NCORES = 8
SCALE = DH ** -0.5  # 0.125
CT = C // 128  # 8 channel tiles
ST = S // 128  # 16 seq tiles
MYROWS = S // 2  # 1024 query rows per core


def build():
    nc = bacc.Bacc("TRN2", target_bir_lowering=False, debug=False,
                   num_devices=NCORES)

    # host-prepared layouts (pure data movement on the host):
    #   xT[c, s] = x[s, c]
    #   wqk[wt, p, ct*128+f] = W_qkv[wt*128+f, ct*128+p]   (Q/K strips)
    #   wv[vch, p, ct*512+f] = W_qkv[2C+vch*512+f, ct*128+p]
    #   wo[et, p, ct*512+e] = W_out[et*512+e, ct*128+p]
    #   bq2d[p, wt] = b_qkv[wt*128+p]
    xT_in = nc.dram_tensor("xT", [C, S], F32R, kind="ExternalInput").ap()
    wqk_in = nc.dram_tensor("wqk", [16, 128, CT * 128], F32R,
                            kind="ExternalInput").ap()
    wv_in = nc.dram_tensor("wv", [2, 128, CT * 512], F32R,
                           kind="ExternalInput").ap()
    wo_in = nc.dram_tensor("wo", [2, 128, CT * 512], F32R,
                           kind="ExternalInput").ap()
    bq2d = nc.dram_tensor("bq2d", [128, 3 * C // 128], F32,
                          kind="ExternalInput").ap()
    b_out = nc.dram_tensor("b_out", [C], F32, kind="ExternalInput").ap()
    out = nc.dram_tensor("out", [MYROWS, C], F32, kind="ExternalOutput").ap()

    # DRAM scratch for the projected tensors (feature-major Q^T/K^T, natural V)
    qT_d = nc.dram_tensor("qT_d", [C, MYROWS], F32R).ap()
    kT_d = nc.dram_tensor("kT_d", [C, S], F32R).ap()
    v_d = nc.dram_tensor("v_d", [S, C], F32R).ap()

    with tile.TileContext(nc) as tc, ExitStack() as ctx:
        const = ctx.enter_context(tc.tile_pool(name="const", bufs=1))
        ident = const.tile([128, 128], F32)
        make_identity(nc, ident[:])  # used for the ones-column writes

        b_sb = const.tile([128, 3 * C // 128], F32)  # b_sb[p, wt] = b_qkv[wt*128+p]
        nc.sync.dma_start(b_sb[:], bq2d)
        bo_sb = const.tile([1, C], F32)
        nc.sync.dma_start(bo_sb[:], b_out[None, :])
        bo_bc = const.tile([128, C], F32)
        nc.gpsimd.partition_broadcast(bo_bc[:], bo_sb[0:1, :])

        persist = ctx.enter_context(tc.tile_pool(name="persist", bufs=1))
        xT = persist.tile([128, CT * S], F32R)  # xT[p, ct*S + s] = x[s, ct*128+p]

        # ---------------- Phase A: QKV projection ----------------
        with ExitStack() as actx:
            wstrip = actx.enter_context(tc.tile_pool(name="wstrip", bufs=3))
            vw_pool = actx.enter_context(tc.tile_pool(name="vw", bufs=2))
            stage = actx.enter_context(tc.tile_pool(name="stage", bufs=4))
            acc_ps = actx.enter_context(
                tc.tile_pool(name="acc_ps", bufs=3, space="PSUM"))

            # x^T resident: 8 c-tiles of [128, S]
            for ct in range(CT):
                nc.sync.dma_start(xT[:, ct * S:(ct + 1) * S],
                                  xT_in[ct * 128:(ct + 1) * 128, :])

            # V natural: rhs = W_v^T chunks [c_part, ct, 512 feats]
            for vch in range(2):
                vw = vw_pool.tile([128, CT * 512], F32R)
                nc.sync.dma_start(vw[:], wv_in[vch])
                for st in range(ST):
                    acc = acc_ps.tile([128, 512], F32)
                    for ct in range(CT):
                        nc.tensor.matmul(
                            acc[:],
                            xT[:, ct * S + st * 128: ct * S + (st + 1) * 128],
                            vw[:, ct * 512:(ct + 1) * 512],
                            start=(ct == 0), stop=(ct == CT - 1))
                    stg = stage.tile([128, 512], F32R)
                    nc.vector.tensor_copy(stg[:], acc[:])
                    nc.sync.dma_start(
                        v_d[st * 128:(st + 1) * 128,
                            vch * 512:(vch + 1) * 512], stg[:])

            # Q^T / K^T: per 128-feature strip, stream W^T column slices
            for wt in (0, 8, 1, 9, 2, 10, 3, 11, 4, 12, 5, 13, 6, 14, 7, 15):
                # ws[p, ct, f] = wT[ct*128+p, wt*128+f]
                ws = wstrip.tile([128, CT * 128], F32R)
                nc.sync.dma_start(ws[:], wqk_in[wt])
                if wt < 8:
                    nsch, dest, drow = 2, qT_d, wt
                else:
                    nsch, dest, drow = 4, kT_d, wt - 8
                for sch in range(nsch):
                    acc = acc_ps.tile([128, 512], F32)
                    for ct in range(CT):
                        nc.tensor.matmul(
                            acc[:],
                            ws[:, ct * 128:(ct + 1) * 128],
                            xT[:, ct * S + sch * 512: ct * S + sch * 512 + 512],
                            start=(ct == 0), stop=(ct == CT - 1))
                    stg = stage.tile([128, 512], F32R)
                    nc.vector.tensor_scalar_add(stg[:], acc[:],
                                                b_sb[:, wt:wt + 1])
                    nc.sync.dma_start(
                        dest[drow * 128:(drow + 1) * 128,
                             sch * 512:(sch + 1) * 512], stg[:])

        # ---------------- Phase B: attention ----------------
        OT = persist.tile([128, CT * MYROWS], F32R)  # OT[p, ct*1024 + i]
        with ExitStack() as bctx:
            kp = bctx.enter_context(tc.tile_pool(name="kp", bufs=2))
            vp = bctx.enter_context(tc.tile_pool(name="vp", bufs=2))
            qp = bctx.enter_context(tc.tile_pool(name="qp", bufs=2))
            pp = bctx.enter_context(tc.tile_pool(name="pp", bufs=2))
            smalls = bctx.enter_context(tc.tile_pool(name="smalls", bufs=3))
            sc_ps = bctx.enter_context(
                tc.tile_pool(name="sc_ps", bufs=1, space="PSUM"))
            pv_ps = bctx.enter_context(
                tc.tile_pool(name="pv_ps", bufs=1, space="PSUM"))

            for hp in range(H // 2):  # head pairs: A = rows 0-63, B = 64-127
                kt = kp.tile([128, S], F32R)
                nc.sync.dma_start(kt[:], kT_d[hp * 128:(hp + 1) * 128, :])
                # pair-wide V tile, padded to 128 stationary columns per
                # (j-tile, head): [V_A(64)|1|0*63|V_B(64)|1|0*63] per j-tile.
                # Full-width weights keep the fp32r fast-weight-load path;
                # the ones column accumulates the softmax denominator at out
                # row 64. One contiguous-chunk 1 MB DMA loads both heads.
                vt = vp.tile([128, ST * 256], F32R)
                vt4 = vt[:].rearrange("p (t g f) -> p t g f", g=2, f=128)
                nc.vector.tensor_scalar(
                    vt4[:, :, :, DH:DH + 1],
                    ident[:, 0:2 * ST].rearrange("p (t g) -> p t g", g=2),
                    0.0, 1.0, mybir.AluOpType.mult, mybir.AluOpType.add)
                for g in range(2):
                    nc.vector.tensor_scalar(
                        vt4[:, :, g:g + 1, DH + 1:128],
                        bo_bc[:, 0:ST * (127 - DH)].rearrange(
                            "p (t g f) -> p t g f", g=1, f=127 - DH),
                        0.0, 0.0, mybir.AluOpType.mult, mybir.AluOpType.mult)
                for g in range(2):
                    nc.sync.dma_start(
                        vt4[:, :, g:g + 1, 0:DH],
                        v_d.rearrange("(t p) (g f) -> p t g f", p=128, f=64)[
                            :, :, 2 * hp + g:2 * hp + g + 1, :])
                for ich in range(2):
                    qt = qp.tile([128, 512], F32R)
                    nc.sync.dma_start(
                        qt[:], qT_d[hp * 128:(hp + 1) * 128,
                                    ich * 512:(ich + 1) * 512])
                    pvs = [pv_ps.tile([128, 512], F32, tag=f"pv{half}",
                                      name=f"pv{half}")
                           for half in range(2)]
                    def emit_pv(pg_prev, js_prev):
                        for half in range(2):
                            for idx, j in enumerate(js_prev):
                                nc.tensor.matmul(
                                    pvs[half][:],
                                    vt[:, j * 256 + half * 128:
                                       j * 256 + half * 128 + 128],
                                    pg_prev[:, (half * 3 + idx) * 512:
                                            (half * 3 + idx + 1) * 512],
                                    start=(j == 0), stop=(j == 15))

                    jb = 0
                    prev = None
                    for blk in (3, 3, 3, 3, 2, 1, 1):
                        js = list(range(jb, jb + blk))
                        jb += blk
                        # one PSUM tile for both heads: A cols [0,1536),
                        # B cols [1536,3072) -> a single exp instruction
                        sc = sc_ps.tile([128, 6 * 512], F32)
                        for idx, j in enumerate(js):
                            # row-packed pair: head A on PE rows 0-63,
                            # head B on rows 64-127, concurrent
                            for half in range(2):
                                p0 = half * 64
                                nc.tensor.matmul(
                                    sc[:, (half * 3 + idx) * 512:
                                       (half * 3 + idx + 1) * 512],
                                    kt[p0:p0 + 64, j * 128:(j + 1) * 128],
                                    qt[p0:p0 + 64, :],
                                    start=True, stop=True)
                        pg = pp.tile([128, 6 * 512], F32R)
                        if blk == 3:
                            nc.scalar.activation(pg[:], sc[:], AF.Exp,
                                                 scale=SCALE)
                        else:
                            for half in range(2):
                                nc.scalar.activation(
                                    pg[:, half * 1536:half * 1536 + blk * 512],
                                    sc[:, half * 1536:half * 1536 + blk * 512],
                                    AF.Exp, scale=SCALE)
                        # software pipeline: PV of the previous block runs
                        # while ACT computes this block's exp
                        if prev is not None:
                            emit_pv(*prev)
                        prev = (pg, js)
                    emit_pv(*prev)
                    for half in range(2):
                        pv = pvs[half]
                        # copy [out|denom] rows out of PSUM immediately so the
                        # accumulator bank frees for the next iteration
                        uv = smalls.tile([65, 512], F32)
                        nc.vector.tensor_copy(uv[:], pv[0:65, :])
                        rec = smalls.tile([1, 512], F32)
                        nc.vector.reciprocal(rec[:], uv[64:65, :])
                        rb = smalls.tile([64, 512], F32)
                        nc.gpsimd.partition_broadcast(rb[:], rec[0:1, :])
                        o2 = smalls.tile([64, 512], F32)
                        nc.vector.tensor_mul(o2[:], uv[0:64, :], rb[:])
                        oslice = OT[half * 64:half * 64 + 64,
                                    hp * MYROWS + ich * 512:
                                    hp * MYROWS + (ich + 1) * 512]
                        nc.vector.tensor_scalar_add(
                            oslice, o2[:],
                            b_sb[half * 64:half * 64 + 64, 16 + hp:17 + hp])

        # ---------------- Phase C: out projection ----------------
        with ExitStack() as cctx:
            woT_pool = cctx.enter_context(tc.tile_pool(name="woT", bufs=2))
            yt_pool = cctx.enter_context(tc.tile_pool(name="yt", bufs=3))
            y_ps = cctx.enter_context(
                tc.tile_pool(name="y_ps", bufs=2, space="PSUM"))

            for et in range(2):
                woT = woT_pool.tile([128, CT * 512], F32R)  # [c_p, ct, 512 e]
                nc.sync.dma_start(woT[:], wo_in[et])
                for it in range(8):
                    y = y_ps.tile([128, 512], F32)
                    for ct in range(CT):
                        nc.tensor.matmul(
                            y[:],
                            OT[:, ct * MYROWS + it * 128: ct * MYROWS + (it + 1) * 128],
                            woT[:, ct * 512:(ct + 1) * 512],
                            start=(ct == 0), stop=(ct == CT - 1))
                    yt = yt_pool.tile([128, 512], F32)
                    nc.vector.tensor_add(yt[:], y[:], bo_bc[:, et * 512:(et + 1) * 512])
                    nc.sync.dma_start(
                        out[it * 128:(it + 1) * 128, et * 512:(et + 1) * 512], yt[:])

    nc.compile()
    return nc


_cache = {}


def _get_nc():
    if "nc" not in _cache:
        _cache["nc"] = build()
    return _cache["nc"]


def kernel(x_q, W_qkv, b_qkv, W_out, b_out):
    """Core c of 8 handles batch c//2, query rows [(c%2)*1024, +1024).

    The per-core x slice is ROLLED by the core's query-row offset so every
    core's own query rows sit at rows [0, MYROWS) of its slice. Attention is
    permutation-invariant over keys, so the rolled K/V ordering does not
    change the output.
    """
    x_q = np.ascontiguousarray(x_q, dtype=np.float32)
    W_qkv = np.ascontiguousarray(W_qkv, dtype=np.float32)
    b_qkv = np.ascontiguousarray(b_qkv, dtype=np.float32)
    W_out = np.ascontiguousarray(W_out, dtype=np.float32)
    b_out = np.ascontiguousarray(b_out, dtype=np.float32)

    nc = _get_nc()
    in_maps = build_in_maps(x_q, W_qkv, b_qkv, W_out, b_out)
    res = run_bass_kernel_spmd(nc, in_maps, list(range(NCORES)))
    out = np.empty((B, S, C), dtype=np.float32)
    for c in range(NCORES):
        b, half = c // 2, c % 2
        out[b, half * MYROWS:(half + 1) * MYROWS] = res.results[c]["out"]
    return out


def build_in_maps(x_q, W_qkv, b_qkv, W_out, b_out):
    x_q = np.ascontiguousarray(x_q, dtype=np.float32)
    W_qkv = np.asarray(W_qkv, dtype=np.float32)
    b_qkv = np.ascontiguousarray(b_qkv, dtype=np.float32)
    W_out = np.asarray(W_out, dtype=np.float32)
    b_out = np.ascontiguousarray(b_out, dtype=np.float32)
    # wqk[wt, p, ct*128+f] = W_qkv[wt*128+f, ct*128+p]
    w4 = W_qkv.reshape(24, 128, CT, 128)            # [wt, f, ct, p]
    wqk = np.ascontiguousarray(w4[:16].transpose(0, 3, 2, 1).reshape(
        16, 128, CT * 128))
    # wv[vch, p, ct*512+f] = W_qkv[2C+vch*512+f, ct*128+p]
    wv5 = W_qkv[2 * C:].reshape(2, 512, CT, 128)    # [vch, f, ct, p]
    wv = np.ascontiguousarray(wv5.transpose(0, 3, 2, 1).reshape(
        2, 128, CT * 512))
    # wo[et, p, ct*512+e] = W_out[et*512+e, ct*128+p]
    wo5 = W_out.reshape(2, 512, CT, 128)            # [et, e, ct, p]
    wo = np.ascontiguousarray(wo5.transpose(0, 3, 2, 1).reshape(
        2, 128, CT * 512))
    bq2d = np.ascontiguousarray(b_qkv.reshape(24, 128).T)
    in_maps = []
    for c in range(NCORES):
        b, half = c // 2, c % 2
        xb = x_q[b]
        if half:
            xb = np.roll(xb, -MYROWS, axis=0)
        in_maps.append({
            "xT": np.ascontiguousarray(xb.T),
            "wqk": wqk,
            "wv": wv,
            "wo": wo,
            "bq2d": bq2d,
            "b_out": b_out,
        })
    return in_maps


if __name__ == "__main__":
    # smoke test with random inputs
    rng = np.random.default_rng(0)
    x_q = rng.standard_normal((B, S, C), dtype=np.float32)
    s = 1.0 / np.sqrt(C)
    W_qkv = rng.uniform(-s, s, (3 * C, C)).astype(np.float32)
    b_qkv = rng.uniform(-s, s, 3 * C).astype(np.float32)
    W_out = rng.uniform(-s, s, (C, C)).astype(np.float32)
    b_out = rng.uniform(-s, s, C).astype(np.float32)
    got = kernel(x_q=x_q, W_qkv=W_qkv, b_qkv=b_qkv, W_out=W_out, b_out=b_out)
    print("smoke ok", got.shape, float(np.abs(got).max()))



# revision 9
# speedup vs baseline: 1.3444x; 1.3444x over previous
"""Multi-head attention (B=4, S=2048, C=1024, H=16) on 8 TRN2 NeuronCores.

Tensor-parallel over heads: core c owns head pair (2c, 2c+1) for ALL 4
batches and computes a partial out-projection y_c = O_c @ W_out[:, c*128:
(c+1)*128].T; the host sums the 8 partials and adds b_out (the TP unshard).

Per core, everything is bf16 (PSUM accumulation in fp32) and SBUF-resident
per batch; x streams in per batch, double-buffered. The per-batch pipeline
keeps ScalarE (exp) saturated while the PE interleaves, as filler between
score/PV blocks, the NEXT batch's QKV projection and the PREVIOUS batch's
out-projection:

  proj(b): qT/kT feature-major [128 pair-feats, 2048 rows] via W-stationary
  matmuls; V keys-major via x-stationary matmuls into a padded stationary
  layout [V_h(64) | 1 | 0*63] per (key-tile, head) so the PV matmul also
  accumulates the softmax denominator at out row 64.

  attention(b, ich of 512 q): per key-tile j: row-packed pair scores
  (head A on PE rows 0-63, head B on 64-127) into a double-buffered 2-bank
  PSUM tile -> one exp (scale=0.125 folded in) -> PV of tile j-1 pipelined
  behind the exp. Denominator rows are collected across the batch via tiny
  SBUF->SBUF DMAs into one [8, 512] tile for a SINGLE reciprocal.

PSUM (8 banks): scores 2x2, PV accumulators 2x1, proj/outproj acc 2x1.
"""

from collections import deque
from contextlib import ExitStack

import numpy as np
import ml_dtypes

import concourse.mybir as mybir
import concourse.tile as tile
from concourse import bacc
from concourse.bass_utils import run_bass_kernel_spmd
from concourse.masks import make_identity

F32 = mybir.dt.float32
BF16 = mybir.dt.bfloat16
AF = mybir.ActivationFunctionType

B, S, C, H, DH = 4, 2048, 1024, 16, 64
NCORES = 8
SCALE = DH ** -0.5  # 0.125
CT = C // 128  # 8 channel tiles
ST = S // 128  # 16 key tiles
NICH = S // 512  # 4 query chunks per batch


def build():
    nc = bacc.Bacc("TRN2", target_bir_lowering=False, debug=False,
                   num_devices=NCORES)

    # host-prepared layouts (pure data movement on the host):
    #   xTd[b, ct, p, s] = x[b, s, ct*128+p]
    #   wq[p, ct*128+f] = W_qkv[c*128+f, ct*128+p]          (this core's Q)
    #   wk / wv same with row offsets 1024+c*128 / 2048+c*128
    #   wo[p, ch] = W_out[ch, c*128+p]
    #   bq/bk/bv[p, 0] = b_qkv[(0|1024|2048) + c*128 + p]
    xTd = nc.dram_tensor("xT", [B, CT, 128, S], BF16, kind="ExternalInput").ap()
    wqd = nc.dram_tensor("wq", [128, CT * 128], BF16, kind="ExternalInput").ap()
    wkd = nc.dram_tensor("wk", [128, CT * 128], BF16, kind="ExternalInput").ap()
    wvd = nc.dram_tensor("wv", [128, CT * 128], BF16, kind="ExternalInput").ap()
    wod = nc.dram_tensor("wo", [128, C], BF16, kind="ExternalInput").ap()
    bqd = nc.dram_tensor("bq", [128, 1], F32, kind="ExternalInput").ap()
    bkd = nc.dram_tensor("bk", [128, 1], F32, kind="ExternalInput").ap()
    bvd = nc.dram_tensor("bv", [128, 1], F32, kind="ExternalInput").ap()
    ypd = nc.dram_tensor("yp", [B * S, C], BF16, kind="ExternalOutput").ap()

    with tile.TileContext(nc) as tc, ExitStack() as ctx:
        const = ctx.enter_context(tc.tile_pool(name="const", bufs=1))
        ident = const.tile([128, 128], F32)
        make_identity(nc, ident[:])
        wsq = const.tile([128, CT * 128], BF16)
        wsk = const.tile([128, CT * 128], BF16)
        wsv = const.tile([128, CT * 128], BF16)
        wo = const.tile([128, C], BF16)
        bq = const.tile([128, 1], F32)
        bk = const.tile([128, 1], F32)
        bv = const.tile([128, 1], F32)
        for dst, src in ((wsq, wqd), (wsk, wkd), (wsv, wvd), (wo, wod),
                         (bq, bqd), (bk, bkd), (bv, bvd)):
            nc.sync.dma_start(dst[:], src)

        # persistent V tiles (ping-pong): ones/zeros pad written once
        vt_pair = [const.tile([128, ST * 256], BF16, name=f"vt{i}")
                   for i in range(2)]
        vt4_pair = []
        for vt in vt_pair:
            vt4 = vt[:].rearrange("p (t g f) -> p t g f", g=2, f=128)
            vt4_pair.append(vt4)
            nc.vector.tensor_scalar(
                vt4[:, :, :, DH:DH + 1],
                ident[:, 0:2 * ST].rearrange("p (t g) -> p t g", g=2),
                0.0, 1.0, mybir.AluOpType.mult, mybir.AluOpType.add)
            for g in range(2):
                nc.vector.tensor_scalar(
                    vt4[:, :, g:g + 1, DH + 1:128],
                    wo[:, 0:ST * (127 - DH)].rearrange(
                        "p (t g f) -> p t g f", g=1, f=127 - DH),
                    0.0, 0.0, mybir.AluOpType.mult, mybir.AluOpType.mult)

        xp = ctx.enter_context(tc.tile_pool(name="xp", bufs=2))
        qp = ctx.enter_context(tc.tile_pool(name="qp", bufs=2))
        kp = ctx.enter_context(tc.tile_pool(name="kp", bufs=2))
        otp = ctx.enter_context(tc.tile_pool(name="otp", bufs=2))
        pgp = ctx.enter_context(tc.tile_pool(name="pgp", bufs=3))
        uvp = ctx.enter_context(tc.tile_pool(name="uvp", bufs=12))
        dnp = ctx.enter_context(tc.tile_pool(name="dnp", bufs=2))
        rcpp = ctx.enter_context(tc.tile_pool(name="rcpp", bufs=2))
        stgp = ctx.enter_context(tc.tile_pool(name="stgp", bufs=4))
        rbp = ctx.enter_context(tc.tile_pool(name="rbp", bufs=4))
        o2p = ctx.enter_context(tc.tile_pool(name="o2p", bufs=4))
        ybp = ctx.enter_context(tc.tile_pool(name="ybp", bufs=4))
        sc_ps = ctx.enter_context(
            tc.tile_pool(name="sc_ps", bufs=2, space="PSUM"))
        pv_ps = ctx.enter_context(
            tc.tile_pool(name="pv_ps", bufs=1, space="PSUM"))
        acc_ps = ctx.enter_context(
            tc.tile_pool(name="acc_ps", bufs=2, space="PSUM"))

        xts = [None] * (B + 1)
        qTs = [None] * B
        kTs = [None] * B
        OTs = [None] * B

        def emit_x_dma(b):
            xt = xp.tile([128, CT * S], BF16)
            xts[b] = xt
            for ct in range(CT):
                nc.sync.dma_start(xt[:, ct * S:(ct + 1) * S], xTd[b, ct])

        # ---- projection of batch b, as a list of PE-sized closures ----
        def make_proj_closures(b):
            xt = xts[b]
            qT = qp.tile([128, S], BF16)
            kT = kp.tile([128, S], BF16)
            qTs[b], kTs[b] = qT, kT
            vt4 = vt4_pair[b % 2]
            cls = []

            # Q/K: per (which, rch) one 8-ct accumulation, split in two
            for wt, dst, bias in ((wsq, qT, bq), (wsk, kT, bk)):
                for rch in range(4):
                    st8 = {}
                    def qk_a(wt=wt, rch=rch, st8=st8):
                        acc = acc_ps.tile([128, 512], F32, name="qkacc", tag="acc")
                        st8["acc"] = acc
                        for ct in range(4):
                            nc.tensor.matmul(
                                acc[:], wt[:, ct * 128:(ct + 1) * 128],
                                xt[:, ct * S + rch * 512: ct * S + rch * 512 + 512],
                                start=(ct == 0), stop=False)
                    def qk_b(wt=wt, dst=dst, bias=bias, rch=rch, st8=st8):
                        acc = st8["acc"]
                        for ct in range(4, 8):
                            nc.tensor.matmul(
                                acc[:], wt[:, ct * 128:(ct + 1) * 128],
                                xt[:, ct * S + rch * 512: ct * S + rch * 512 + 512],
                                start=False, stop=(ct == 7))
                        nc.vector.tensor_scalar_add(
                            dst[:, rch * 512:(rch + 1) * 512], acc[:],
                            bias[:, 0:1])
                    cls.append(qk_a)
                    cls.append(qk_b)

            # V: groups of 4 key-tiles sharing one [128, 512] psum tile
            for g4 in range(4):
                st8 = {}
                for u in range(4):
                    def v_u(g4=g4, u=u, st8=st8):
                        if u == 0:
                            st8["acc"] = acc_ps.tile([128, 512], F32,
                                                     name="vacc", tag="acc")
                        acc = st8["acc"]
                        kt = g4 * 4 + u
                        for ct in range(CT):
                            nc.tensor.matmul(
                                acc[:, u * 128:(u + 1) * 128],
                                xt[:, ct * S + kt * 128: ct * S + (kt + 1) * 128],
                                wsv[:, ct * 128:(ct + 1) * 128],
                                start=(ct == 0), stop=(ct == CT - 1))
                        if u == 3:
                            nc.vector.tensor_copy(
                                vt4[:, g4 * 4:(g4 + 1) * 4, :, 0:DH],
                                acc[:].rearrange("p (u g f) -> p u g f",
                                                 u=4, f=DH))
                    cls.append(v_u)
            return cls

        # ---- out-projection of batch b as closures ----
        def make_outproj_closures(b):
            OT = OTs[b]
            cls = []
            for qs in range(16):
                for et in range(2):
                    def y_u(b=b, qs=qs, et=et, OT=OT):
                        acc = acc_ps.tile([128, 512], F32, name="yacc", tag="acc")
                        nc.tensor.matmul(
                            acc[:], OT[:, qs * 128:(qs + 1) * 128],
                            wo[:, et * 512:(et + 1) * 512],
                            start=True, stop=True)
                        yb = ybp.tile([128, 512], BF16, name="yb")
                        nc.vector.tensor_copy(yb[:], acc[:])
                        nc.sync.dma_start(
                            ypd[b * S + qs * 128: b * S + (qs + 1) * 128,
                                et * 512:(et + 1) * 512], yb[:])
                    cls.append(y_u)
            return cls

        fillers = deque()

        def run_filler(n):
            for _ in range(n):
                if fillers:
                    fillers.popleft()()

        # ---- attention for batch b ----
        def attention(b):
            qT, kT = qTs[b], kTs[b]
            vt4 = vt4_pair[b % 2]
            OT = otp.tile([128, S], BF16)
            OTs[b] = OT
            dcol = dnp.tile([8, 512], F32)
            uvs = {}
            for ich in range(NICH):
                pvs = [pv_ps.tile([128, 512], F32, tag=f"pv{h}",
                                  name=f"pv{h}") for h in range(2)]
                prev = None
                for j in range(ST):
                    sc = sc_ps.tile([128, 1024], F32)
                    for half in range(2):
                        p0 = half * 64
                        nc.tensor.matmul(
                            sc[:, half * 512:(half + 1) * 512],
                            kT[p0:p0 + 64, j * 128:(j + 1) * 128],
                            qT[p0:p0 + 64, ich * 512:(ich + 1) * 512],
                            start=True, stop=True)
                    pg = pgp.tile([128, 1024], BF16)
                    nc.scalar.activation(pg[:], sc[:], AF.Exp, scale=SCALE)
                    if prev is not None:
                        pj, ppg = prev
                        for half in range(2):
                            nc.tensor.matmul(
                                pvs[half][:], vt4[:, pj, half, :],
                                ppg[:, half * 512:(half + 1) * 512],
                                start=(pj == 0), stop=False)
                    prev = (j, pg)
                    run_filler(1)
                pj, ppg = prev
                for half in range(2):
                    nc.tensor.matmul(
                        pvs[half][:], vt4[:, pj, half, :],
                        ppg[:, half * 512:(half + 1) * 512],
                        start=False, stop=True)
                for half in range(2):
                    uv = uvp.tile([65, 512], F32)
                    nc.vector.tensor_copy(uv[:], pvs[half][0:65, :])
                    nc.sync.dma_start(dcol[2 * ich + half:2 * ich + half + 1, :],
                                      uv[64:65, :])
                    uvs[(ich, half)] = uv
                run_filler(2)

            # single reciprocal for all 8 denominator rows of the batch;
            # partition_broadcast needs a partition-0 source, so stage each
            # row back to partition 0 of a [1, 4096] strip first
            rcp = rcpp.tile([8, 512], F32)
            nc.vector.reciprocal(rcp[:], dcol[:])
            for ich in range(NICH):
                for half in range(2):
                    k = 2 * ich + half
                    stg = stgp.tile([1, 512], F32, name="stg")
                    nc.sync.dma_start(stg[0:1, :], rcp[k:k + 1, :])
                    rb = rbp.tile([64, 512], F32)
                    nc.gpsimd.partition_broadcast(rb[:], stg[0:1, :])
                    uv = uvs[(ich, half)]
                    o2 = o2p.tile([64, 512], F32)
                    nc.vector.tensor_mul(o2[:], uv[0:64, :], rb[:])
                    nc.vector.tensor_scalar_add(
                        OT[half * 64:half * 64 + 64,
                           ich * 512:(ich + 1) * 512],
                        o2[:], bv[half * 64:half * 64 + 64, 0:1])

        # ================= emission =================
        # prefetch x of batches 0 and 1 before batch 0's filler closures can
        # reach the PE FIFO, so proj(1) never stalls the attention(0) stream
        emit_x_dma(0)
        emit_x_dma(1)
        for cl in make_proj_closures(0):
            cl()
        for b in range(B):
            if b + 1 < B:
                if b + 2 < B:
                    emit_x_dma(b + 2)
                fillers.extend(make_proj_closures(b + 1))
            attention(b)
            fillers.extend(make_outproj_closures(b))
            if b + 1 == B:
                run_filler(len(fillers))

    nc.compile()
    return nc


_cache = {}


def _get_nc():
    if "nc" not in _cache:
        _cache["nc"] = build()
    return _cache["nc"]


def build_in_maps(x_q, W_qkv, b_qkv, W_out, b_out):
    x_q = np.ascontiguousarray(x_q, dtype=np.float32)
    W_qkv = np.asarray(W_qkv, dtype=np.float32)
    b_qkv = np.ascontiguousarray(b_qkv, dtype=np.float32)
    W_out = np.asarray(W_out, dtype=np.float32)
    bf = ml_dtypes.bfloat16
    # xTd[b, ct, p, s] = x[b, s, ct*128+p]
    xT = np.ascontiguousarray(
        x_q.transpose(0, 2, 1).reshape(B, CT, 128, S)).astype(bf)
    in_maps = []
    for c in range(NCORES):
        def wslice(off):
            # [p, ct*128+f] = W_qkv[off + c*128 + f, ct*128 + p]
            sl = W_qkv[off + c * 128: off + (c + 1) * 128, :]  # [f, chan]
            return np.ascontiguousarray(
                sl.reshape(128, CT, 128).transpose(2, 1, 0).reshape(
                    128, CT * 128)).astype(bf)
        wo = np.ascontiguousarray(W_out[:, c * 128:(c + 1) * 128].T).astype(bf)
        in_maps.append({
            "xT": xT,
            "wq": wslice(0),
            "wk": wslice(C),
            "wv": wslice(2 * C),
            "wo": wo,
            "bq": np.ascontiguousarray(
                b_qkv[c * 128:(c + 1) * 128].reshape(128, 1)),
            "bk": np.ascontiguousarray(
                b_qkv[C + c * 128: C + (c + 1) * 128].reshape(128, 1)),
            "bv": np.ascontiguousarray(
                b_qkv[2 * C + c * 128: 2 * C + (c + 1) * 128].reshape(128, 1)),
        })
    return in_maps


def kernel(x_q, W_qkv, b_qkv, W_out, b_out):
    """Core c computes heads (2c, 2c+1) for all batches and the partial
    out-projection against W_out[:, c*128:(c+1)*128]; the host sums the
    8 partials and adds b_out (the tensor-parallel unshard)."""
    b_out = np.ascontiguousarray(b_out, dtype=np.float32)
    nc = _get_nc()
    in_maps = build_in_maps(x_q, W_qkv, b_qkv, W_out, b_out)
    res = run_bass_kernel_spmd(nc, in_maps, list(range(NCORES)))
    y = np.zeros((B * S, C), dtype=np.float32)
    for c in range(NCORES):
        y += np.asarray(res.results[c]["yp"]).astype(np.float32)
    y += b_out[None, :]
    return y.reshape(B, S, C)


if __name__ == "__main__":
    rng = np.random.default_rng(0)
    x_q = rng.standard_normal((B, S, C), dtype=np.float32)
    s = 1.0 / np.sqrt(C)
    W_qkv = rng.uniform(-s, s, (3 * C, C)).astype(np.float32)
    b_qkv = rng.uniform(-s, s, 3 * C).astype(np.float32)
    W_out = rng.uniform(-s, s, (C, C)).astype(np.float32)
    b_out = rng.uniform(-s, s, C).astype(np.float32)
    got = kernel(x_q=x_q, W_qkv=W_qkv, b_qkv=b_qkv, W_out=W_out, b_out=b_out)
    print("smoke ok", got.shape, float(np.abs(got).max()))


# revision 14
# speedup vs baseline: 1.4333x; 1.0661x over previous
"""Multi-head attention (B=4, S=2048, C=1024, H=16) on 8 TRN2 NeuronCores.

Tensor-parallel over heads: core c owns head pair (2c, 2c+1) for ALL 4
batches and computes a partial out-projection y_c = O_c @ W_out[:, c*128:
(c+1)*128].T; the host sums the 8 partials and adds b_out (the TP unshard).

Per core, everything is bf16 (PSUM accumulation in fp32) and SBUF-resident
per batch; x streams in per batch, double-buffered. The per-batch pipeline
keeps ScalarE (exp) saturated while the PE interleaves, as filler between
score/PV blocks, the NEXT batch's QKV projection and the PREVIOUS batch's
out-projection:

  proj(b): qT/kT feature-major [128 pair-feats, 2048 rows] via W-stationary
  matmuls; V keys-major via x-stationary matmuls into a padded stationary
  layout [V_h(64) | 1 | 0*63] per (key-tile, head) so the PV matmul also
  accumulates the softmax denominator at out row 64.

  attention(b, ich of 512 q): per key-tile j: row-packed pair scores
  (head A on PE rows 0-63, head B on 64-127) into a double-buffered 2-bank
  PSUM tile -> one exp (scale=0.125 folded in) -> PV of tile j-1 pipelined
  behind the exp. Denominator rows are collected across the batch via tiny
  SBUF->SBUF DMAs into one [8, 512] tile for a SINGLE reciprocal.

PSUM (8 banks): scores 2x2, PV accumulators 2x1, proj/outproj acc 2x1.
"""

from collections import deque
from contextlib import ExitStack

import numpy as np
import ml_dtypes

import concourse.mybir as mybir
import concourse.tile as tile
from concourse import bacc
from concourse.bass_utils import run_bass_kernel_spmd
from concourse.masks import make_identity

F32 = mybir.dt.float32
BF16 = mybir.dt.bfloat16
AF = mybir.ActivationFunctionType

B, S, C, H, DH = 4, 2048, 1024, 16, 64
NCORES = 8
SCALE = DH ** -0.5  # 0.125
CT = C // 128  # 8 channel tiles
ST = S // 128  # 16 key tiles
NICH = S // 512  # 4 query chunks per batch


def build():
    nc = bacc.Bacc("TRN2", target_bir_lowering=False, debug=False,
                   num_devices=NCORES)

    # host-prepared layouts (pure data movement on the host):
    #   xTd[b, ct, p, s] = x[b, s, ct*128+p]
    #   wq[p, ct*128+f] = W_qkv[c*128+f, ct*128+p]          (this core's Q)
    #   wk / wv same with row offsets 1024+c*128 / 2048+c*128
    #   wo[p, ch] = W_out[ch, c*128+p]
    #   bq/bk/bv[p, 0] = b_qkv[(0|1024|2048) + c*128 + p]
    xTd = nc.dram_tensor("xT", [B, CT, 128, S], BF16, kind="ExternalInput").ap()
    wqd = nc.dram_tensor("wq", [128, CT * 128], BF16, kind="ExternalInput").ap()
    wkd = nc.dram_tensor("wk", [128, CT * 128], BF16, kind="ExternalInput").ap()
    wvd = nc.dram_tensor("wv", [128, CT * 128], BF16, kind="ExternalInput").ap()
    wod = nc.dram_tensor("wo", [128, C], BF16, kind="ExternalInput").ap()
    bqd = nc.dram_tensor("bq", [128, 1], F32, kind="ExternalInput").ap()
    bkd = nc.dram_tensor("bk", [128, 1], F32, kind="ExternalInput").ap()
    bvd = nc.dram_tensor("bv", [128, 1], F32, kind="ExternalInput").ap()
    ypd = nc.dram_tensor("yp", [B * S, C], BF16, kind="ExternalOutput").ap()

    with tile.TileContext(nc) as tc, ExitStack() as ctx:
        const = ctx.enter_context(tc.tile_pool(name="const", bufs=1))
        ident = const.tile([128, 128], F32)
        make_identity(nc, ident[:])
        wsq = const.tile([128, CT * 128], BF16)
        wsk = const.tile([128, CT * 128], BF16)
        wsv = const.tile([128, CT * 128], BF16)
        wo = const.tile([128, C], BF16)
        bq = const.tile([128, 1], F32)
        bk = const.tile([128, 1], F32)
        bv = const.tile([128, 1], F32)
        for dst, src in ((wsq, wqd), (wsk, wkd), (wsv, wvd), (wo, wod),
                         (bq, bqd), (bk, bkd), (bv, bvd)):
            nc.sync.dma_start(dst[:], src)

        # persistent V tiles (ping-pong): ones/zeros pad written once
        vt_pair = [const.tile([128, ST * 256], BF16, name=f"vt{i}")
                   for i in range(2)]
        vt4_pair = []
        for vt in vt_pair:
            vt4 = vt[:].rearrange("p (t g f) -> p t g f", g=2, f=128)
            vt4_pair.append(vt4)
            nc.vector.tensor_scalar(
                vt4[:, :, :, DH:DH + 1],
                ident[:, 0:2 * ST].rearrange("p (t g) -> p t g", g=2),
                0.0, 1.0, mybir.AluOpType.mult, mybir.AluOpType.add)
            for g in range(2):
                nc.vector.tensor_scalar(
                    vt4[:, :, g:g + 1, DH + 1:128],
                    wo[:, 0:ST * (127 - DH)].rearrange(
                        "p (t g f) -> p t g f", g=1, f=127 - DH),
                    0.0, 0.0, mybir.AluOpType.mult, mybir.AluOpType.mult)

        xp = ctx.enter_context(tc.tile_pool(name="xp", bufs=2))
        qp = ctx.enter_context(tc.tile_pool(name="qp", bufs=2))
        kp = ctx.enter_context(tc.tile_pool(name="kp", bufs=2))
        otp = ctx.enter_context(tc.tile_pool(name="otp", bufs=2))
        pgp = ctx.enter_context(tc.tile_pool(name="pgp", bufs=3))
        uvp = ctx.enter_context(tc.tile_pool(name="uvp", bufs=8))
        dnp = ctx.enter_context(tc.tile_pool(name="dnp", bufs=4))
        rcpp = ctx.enter_context(tc.tile_pool(name="rcpp", bufs=4))
        stgp = ctx.enter_context(tc.tile_pool(name="stgp", bufs=4))
        rbp = ctx.enter_context(tc.tile_pool(name="rbp", bufs=4))
        o2p = ctx.enter_context(tc.tile_pool(name="o2p", bufs=4))
        ybp = ctx.enter_context(tc.tile_pool(name="ybp", bufs=4))
        sc_ps = ctx.enter_context(
            tc.tile_pool(name="sc_ps", bufs=2, space="PSUM"))
        pv_ps = ctx.enter_context(
            tc.tile_pool(name="pv_ps", bufs=1, space="PSUM"))
        acc_ps = ctx.enter_context(
            tc.tile_pool(name="acc_ps", bufs=2, space="PSUM"))

        xts = [None] * (B + 1)
        qTs = [None] * B
        kTs = [None] * B
        OTs = [None] * B

        def emit_x_dma(b):
            xt = xp.tile([128, CT * S], BF16)
            xts[b] = xt
            for ct in range(CT):
                nc.sync.dma_start(xt[:, ct * S:(ct + 1) * S], xTd[b, ct])

        # ---- projection of batch b, as lists of PE-sized closures ----
        def make_proj_closures(b):
            xt = xts[b]
            qT = qp.tile([128, S], BF16)
            kT = kp.tile([128, S], BF16)
            qTs[b], kTs[b] = qT, kT
            vt4 = vt4_pair[b % 2]

            # Q/K: per (which, rch) one 8-ct accumulation, split into 4
            # ~0.45us closures so the filler pacing stays fine-grained
            def qk_quads(wt, dst, bias, rch):
                st8 = {}
                def qk_u(q4):
                    def f():
                        if q4 == 0:
                            st8["acc"] = acc_ps.tile([128, 512], F32,
                                                     name="qkacc", tag="acc")
                        acc = st8["acc"]
                        for ct in range(2 * q4, 2 * q4 + 2):
                            nc.tensor.matmul(
                                acc[:], wt[:, ct * 128:(ct + 1) * 128],
                                xt[:, ct * S + rch * 512: ct * S + rch * 512 + 512],
                                start=(ct == 0), stop=(ct == 7))
                        if q4 == 3:
                            nc.vector.tensor_scalar_add(
                                dst[:, rch * 512:(rch + 1) * 512], acc[:],
                                bias[:, 0:1])
                    return f
                return [qk_u(q4) for q4 in range(4)]

            k_cls = [c for rch in range(4) for c in qk_quads(wsk, kT, bk, rch)]
            q_cls = [qk_quads(wsq, qT, bq, rch) for rch in range(4)]

            # V: per group of 4 key-tiles, 8 closures of 4-ct halves
            def v_group(g4):
                st8 = {}
                def v_u(u, h):
                    def f():
                        if u == 0 and h == 0:
                            st8["acc"] = acc_ps.tile([128, 512], F32,
                                                     name="vacc", tag="acc")
                        acc = st8["acc"]
                        kt = g4 * 4 + u
                        for ct in range(4 * h, 4 * h + 4):
                            nc.tensor.matmul(
                                acc[:, u * 128:(u + 1) * 128],
                                xt[:, ct * S + kt * 128: ct * S + (kt + 1) * 128],
                                wsv[:, ct * 128:(ct + 1) * 128],
                                start=(ct == 0), stop=(ct == CT - 1))
                        if u == 3 and h == 1:
                            nc.vector.tensor_copy(
                                vt4[:, g4 * 4:(g4 + 1) * 4, :, 0:DH],
                                acc[:].rearrange("p (u g f) -> p u g f",
                                                 u=4, f=DH))
                    return f
                return [v_u(u, h) for u in range(4) for h in range(2)]

            v_cls = [v_group(g4) for g4 in range(4)]
            return k_cls, q_cls, v_cls

        # ---- out-projection of one ich of batch b as weighted closures ----
        def make_outproj_closures(b, ich):
            OT = OTs[b]
            cls = []
            for qs in range(4 * ich, 4 * ich + 4):
                for et in range(2):
                    def y_u(b=b, qs=qs, et=et, OT=OT):
                        acc = acc_ps.tile([128, 512], F32, name="yacc", tag="acc")
                        nc.tensor.matmul(
                            acc[:], OT[:, qs * 128:(qs + 1) * 128],
                            wo[:, et * 512:(et + 1) * 512],
                            start=True, stop=True)
                        yb = ybp.tile([128, 512], BF16, name="yb")
                        nc.vector.tensor_copy(yb[:], acc[:])
                        nc.sync.dma_start(
                            ypd[b * S + qs * 128: b * S + (qs + 1) * 128,
                                et * 512:(et + 1) * 512], yb[:])
                    cls.append(y_u)
            return cls

        fillers = deque()

        def run_filler(n):
            for _ in range(n):
                if fillers:
                    fillers.popleft()()

        def drain_filler():
            while fillers:
                fillers.popleft()()

        # ---- attention for batch b ----
        def attention(b, slots=1):
            qT, kT = qTs[b], kTs[b]
            vt4 = vt4_pair[b % 2]
            OT = otp.tile([128, S], BF16)
            OTs[b] = OT
            for ich in range(NICH):
                pvs = [pv_ps.tile([128, 512], F32, tag=f"pv{h}",
                                  name=f"pv{h}") for h in range(2)]
                prev = None
                for j in range(ST):
                    sc = sc_ps.tile([128, 1024], F32)
                    for half in range(2):
                        p0 = half * 64
                        nc.tensor.matmul(
                            sc[:, half * 512:(half + 1) * 512],
                            kT[p0:p0 + 64, j * 128:(j + 1) * 128],
                            qT[p0:p0 + 64, ich * 512:(ich + 1) * 512],
                            start=True, stop=True)
                    pg = pgp.tile([128, 1024], BF16)
                    nc.scalar.activation(pg[:], sc[:], AF.Exp, scale=SCALE)
                    if prev is not None:
                        pj, ppg = prev
                        for half in range(2):
                            nc.tensor.matmul(
                                pvs[half][:], vt4[:, pj, half, :],
                                ppg[:, half * 512:(half + 1) * 512],
                                start=(pj == 0), stop=False)
                    prev = (j, pg)
                    run_filler(slots)
                pj, ppg = prev
                for half in range(2):
                    nc.tensor.matmul(
                        pvs[half][:], vt4[:, pj, half, :],
                        ppg[:, half * 512:(half + 1) * 512],
                        start=False, stop=True)
                # per-ich normalize: collect this ich's two denominator rows,
                # one cheap [2,512] reciprocal, broadcast, scale, bias
                dcol2 = dnp.tile([2, 512], F32, name="dcol2")
                uvs2 = []
                for half in range(2):
                    uv = uvp.tile([65, 512], F32, name="uv")
                    nc.vector.tensor_copy(uv[:], pvs[half][0:65, :])
                    nc.sync.dma_start(dcol2[half:half + 1, :], uv[64:65, :])
                    uvs2.append(uv)
                rcp2 = rcpp.tile([2, 512], F32, name="rcp2")
                nc.vector.reciprocal(rcp2[:], dcol2[:])
                for half in range(2):
                    stg = stgp.tile([1, 512], F32, name="stg")
                    nc.sync.dma_start(stg[0:1, :], rcp2[half:half + 1, :])
                    rb = rbp.tile([64, 512], F32)
                    nc.gpsimd.partition_broadcast(rb[:], stg[0:1, :])
                    o2 = o2p.tile([64, 512], F32)
                    nc.vector.tensor_mul(o2[:], uvs2[half][0:64, :], rb[:])
                    nc.vector.tensor_scalar_add(
                        OT[half * 64:half * 64 + 64,
                           ich * 512:(ich + 1) * 512],
                        o2[:], bv[half * 64:half * 64 + 64, 0:1])
                # out-projection for the PREVIOUS ich (lag keeps the PE from
                # reaching a y matmul before its OT chunk is normalized)
                if ich >= 1:
                    fillers.extend(make_outproj_closures(b, ich - 1))
                run_filler(8 * slots)
            fillers.extend(make_outproj_closures(b, NICH - 1))

        # ================= emission =================
        # prefetch x of batches 0 and 1 before batch 0's filler closures can
        # reach the PE FIFO, so proj(1) never stalls the attention(0) stream
        emit_x_dma(0)
        emit_x_dma(1)
        # fast start: only K, Q(rch0) and V(group 0) inline; the rest of
        # proj(0) becomes priority filler consumed by attention(0) in an
        # order matching when attention first needs each piece
        k0, q0, v0 = make_proj_closures(0)
        for cl in k0 + q0[0] + v0[0]:
            cl()
        fillers.extend(v0[1] + v0[2] + v0[3] + q0[1] + q0[2] + q0[3])
        for b in range(B):
            if b + 1 < B:
                if b + 2 < B:
                    emit_x_dma(b + 2)
                kc, qc, vc = make_proj_closures(b + 1)
                fillers.extend(kc + [c for r in qc for c in r]
                               + [c for g in vc for c in g])
            assert len(fillers) <= (100 if b == 0 else 80), (b, len(fillers))
            attention(b, slots=2 if b == 0 else 1)
        drain_filler()

    nc.compile()
    return nc


_cache = {}


def _get_nc():
    if "nc" not in _cache:
        _cache["nc"] = build()
    return _cache["nc"]


def build_in_maps(x_q, W_qkv, b_qkv, W_out, b_out):
    x_q = np.ascontiguousarray(x_q, dtype=np.float32)
    W_qkv = np.asarray(W_qkv, dtype=np.float32)
    b_qkv = np.ascontiguousarray(b_qkv, dtype=np.float32)
    W_out = np.asarray(W_out, dtype=np.float32)
    bf = ml_dtypes.bfloat16
    # xTd[b, ct, p, s] = x[b, s, ct*128+p]
    xT = np.ascontiguousarray(
        x_q.transpose(0, 2, 1).reshape(B, CT, 128, S)).astype(bf)
    in_maps = []
    for c in range(NCORES):
        def wslice(off):
            # [p, ct*128+f] = W_qkv[off + c*128 + f, ct*128 + p]
            sl = W_qkv[off + c * 128: off + (c + 1) * 128, :]  # [f, chan]
            return np.ascontiguousarray(
                sl.reshape(128, CT, 128).transpose(2, 1, 0).reshape(
                    128, CT * 128)).astype(bf)
        wo = np.ascontiguousarray(W_out[:, c * 128:(c + 1) * 128].T).astype(bf)
        in_maps.append({
            "xT": xT,
            "wq": wslice(0),
            "wk": wslice(C),
            "wv": wslice(2 * C),
            "wo": wo,
            "bq": np.ascontiguousarray(
                b_qkv[c * 128:(c + 1) * 128].reshape(128, 1)),
            "bk": np.ascontiguousarray(
                b_qkv[C + c * 128: C + (c + 1) * 128].reshape(128, 1)),
            "bv": np.ascontiguousarray(
                b_qkv[2 * C + c * 128: 2 * C + (c + 1) * 128].reshape(128, 1)),
        })
    return in_maps


def kernel(x_q, W_qkv, b_qkv, W_out, b_out):
    """Core c computes heads (2c, 2c+1) for all batches and the partial
    out-projection against W_out[:, c*128:(c+1)*128]; the host sums the
    8 partials and adds b_out (the tensor-parallel unshard)."""
    b_out = np.ascontiguousarray(b_out, dtype=np.float32)
    nc = _get_nc()
    in_maps = build_in_maps(x_q, W_qkv, b_qkv, W_out, b_out)
    res = run_bass_kernel_spmd(nc, in_maps, list(range(NCORES)))
    y = np.zeros((B * S, C), dtype=np.float32)
    for c in range(NCORES):
        y += np.asarray(res.results[c]["yp"]).astype(np.float32)
    y += b_out[None, :]
    return y.reshape(B, S, C)


if __name__ == "__main__":
    rng = np.random.default_rng(0)
    x_q = rng.standard_normal((B, S, C), dtype=np.float32)
    s = 1.0 / np.sqrt(C)
    W_qkv = rng.uniform(-s, s, (3 * C, C)).astype(np.float32)
    b_qkv = rng.uniform(-s, s, 3 * C).astype(np.float32)
    W_out = rng.uniform(-s, s, (C, C)).astype(np.float32)
    b_out = rng.uniform(-s, s, C).astype(np.float32)
    got = kernel(x_q=x_q, W_qkv=W_qkv, b_qkv=b_qkv, W_out=W_out, b_out=b_out)
    print("smoke ok", got.shape, float(np.abs(got).max()))


# revision 16
# speedup vs baseline: 1.4398x; 1.0045x over previous
"""Multi-head attention (B=4, S=2048, C=1024, H=16) on 8 TRN2 NeuronCores.

Tensor-parallel over heads: core c owns head pair (2c, 2c+1) for ALL 4
batches and computes a partial out-projection y_c = O_c @ W_out[:, c*128:
(c+1)*128].T; the host sums the 8 partials and adds b_out (the TP unshard).

Per core, everything is bf16 (PSUM accumulation in fp32) and SBUF-resident
per batch; x streams in per batch, double-buffered. The per-batch pipeline
keeps ScalarE (exp) saturated while the PE interleaves, as filler between
score/PV blocks, the NEXT batch's QKV projection and the PREVIOUS batch's
out-projection:

  proj(b): qT/kT feature-major [128 pair-feats, 2048 rows] via W-stationary
  matmuls; V keys-major via x-stationary matmuls into a padded stationary
  layout [V_h(64) | 1 | 0*63] per (key-tile, head) so the PV matmul also
  accumulates the softmax denominator at out row 64.

  attention(b, ich of 512 q): per key-tile j: row-packed pair scores
  (head A on PE rows 0-63, head B on 64-127) into a double-buffered 2-bank
  PSUM tile -> one exp (scale=0.125 folded in) -> PV of tile j-1 pipelined
  behind the exp. Denominator rows are collected across the batch via tiny
  SBUF->SBUF DMAs into one [8, 512] tile for a SINGLE reciprocal.

PSUM (8 banks): scores 2x2, PV accumulators 2x1, proj/outproj acc 2x1.
"""

from collections import deque
from contextlib import ExitStack

import numpy as np
import ml_dtypes

import concourse.mybir as mybir
import concourse.tile as tile
from concourse import bacc
from concourse.bass_utils import run_bass_kernel_spmd
from concourse.masks import make_identity

F32 = mybir.dt.float32
BF16 = mybir.dt.bfloat16
AF = mybir.ActivationFunctionType

B, S, C, H, DH = 4, 2048, 1024, 16, 64
NCORES = 8
SCALE = DH ** -0.5  # 0.125
CT = C // 128  # 8 channel tiles
ST = S // 128  # 16 key tiles
NICH = S // 512  # 4 query chunks per batch


def build():
    nc = bacc.Bacc("TRN2", target_bir_lowering=False, debug=False,
                   num_devices=NCORES)

    # host-prepared layouts (pure data movement on the host):
    #   xTd[b, ct, p, s] = x[b, s, ct*128+p]
    #   wq[p, ct*128+f] = W_qkv[c*128+f, ct*128+p]          (this core's Q)
    #   wk / wv same with row offsets 1024+c*128 / 2048+c*128
    #   wo[p, ch] = W_out[ch, c*128+p]
    #   bq/bk/bv[p, 0] = b_qkv[(0|1024|2048) + c*128 + p]
    xTd = nc.dram_tensor("xT", [B, CT, 128, S], BF16, kind="ExternalInput").ap()
    wqd = nc.dram_tensor("wq", [128, CT * 128], BF16, kind="ExternalInput").ap()
    wkd = nc.dram_tensor("wk", [128, CT * 128], BF16, kind="ExternalInput").ap()
    wvd = nc.dram_tensor("wv", [128, CT * 128], BF16, kind="ExternalInput").ap()
    wod = nc.dram_tensor("wo", [128, C], BF16, kind="ExternalInput").ap()
    bqd = nc.dram_tensor("bq", [128, 1], F32, kind="ExternalInput").ap()
    bkd = nc.dram_tensor("bk", [128, 1], F32, kind="ExternalInput").ap()
    bvd = nc.dram_tensor("bv", [128, 1], F32, kind="ExternalInput").ap()
    ypd = nc.dram_tensor("yp", [B * S, C], BF16, kind="ExternalOutput").ap()

    with tile.TileContext(nc) as tc, ExitStack() as ctx:
        const = ctx.enter_context(tc.tile_pool(name="const", bufs=1))
        ident = const.tile([128, 128], F32)
        make_identity(nc, ident[:])
        wsq = const.tile([128, CT * 128], BF16)
        wsk = const.tile([128, CT * 128], BF16)
        wsv = const.tile([128, CT * 128], BF16)
        wo = const.tile([128, C], BF16)
        bq = const.tile([128, 1], F32)
        bk = const.tile([128, 1], F32)
        bv = const.tile([128, 1], F32)
        for dst, src in ((wsk, wkd), (wsq, wqd), (bq, bqd), (bk, bkd)):
            nc.sync.dma_start(dst[:], src)

        # warmup tile for PE clock ramp during the startup DMA window
        wrm = const.tile([128, 512], BF16)
        for i in range(4):
            nc.vector.tensor_scalar(
                wrm[:, i * 128:(i + 1) * 128], ident[:, 0:128],
                0.0, 0.0, mybir.AluOpType.mult, mybir.AluOpType.mult)

        # persistent V tiles (ping-pong): ones/zeros pad written once
        vt_pair = [const.tile([128, ST * 256], BF16, name=f"vt{i}")
                   for i in range(2)]
        vt4_pair = []
        for vt in vt_pair:
            vt4 = vt[:].rearrange("p (t g f) -> p t g f", g=2, f=128)
            vt4_pair.append(vt4)
            nc.vector.tensor_scalar(
                vt4[:, :, :, DH:DH + 1],
                ident[:, 0:2 * ST].rearrange("p (t g) -> p t g", g=2),
                0.0, 1.0, mybir.AluOpType.mult, mybir.AluOpType.add)
            for g in range(2):
                nc.vector.tensor_scalar(
                    vt4[:, :, g:g + 1, DH + 1:128],
                    wsk[:, 0:ST * (127 - DH)].rearrange(
                        "p (t g f) -> p t g f", g=1, f=127 - DH),
                    0.0, 0.0, mybir.AluOpType.mult, mybir.AluOpType.mult)

        xp = ctx.enter_context(tc.tile_pool(name="xp", bufs=2))
        qp = ctx.enter_context(tc.tile_pool(name="qp", bufs=2))
        kp = ctx.enter_context(tc.tile_pool(name="kp", bufs=2))
        otp = ctx.enter_context(tc.tile_pool(name="otp", bufs=2))
        pgp = ctx.enter_context(tc.tile_pool(name="pgp", bufs=3))
        uvp = ctx.enter_context(tc.tile_pool(name="uvp", bufs=8))
        dnp = ctx.enter_context(tc.tile_pool(name="dnp", bufs=4))
        rcpp = ctx.enter_context(tc.tile_pool(name="rcpp", bufs=4))
        stgp = ctx.enter_context(tc.tile_pool(name="stgp", bufs=4))
        rbp = ctx.enter_context(tc.tile_pool(name="rbp", bufs=4))
        o2p = ctx.enter_context(tc.tile_pool(name="o2p", bufs=4))
        ybp = ctx.enter_context(tc.tile_pool(name="ybp", bufs=4))
        sc_ps = ctx.enter_context(
            tc.tile_pool(name="sc_ps", bufs=2, space="PSUM"))
        pv_ps = ctx.enter_context(
            tc.tile_pool(name="pv_ps", bufs=1, space="PSUM"))
        acc_ps = ctx.enter_context(
            tc.tile_pool(name="acc_ps", bufs=2, space="PSUM"))

        xts = [None] * (B + 1)
        qTs = [None] * B
        kTs = [None] * B
        OTs = [None] * B

        def emit_x_dma(b):
            xt = xp.tile([128, CT * S], BF16)
            xts[b] = xt
            for ct in range(CT):
                nc.sync.dma_start(xt[:, ct * S:(ct + 1) * S], xTd[b, ct])

        # ---- projection of batch b, as lists of PE-sized closures ----
        def make_proj_closures(b):
            xt = xts[b]
            qT = qp.tile([128, S], BF16)
            kT = kp.tile([128, S], BF16)
            qTs[b], kTs[b] = qT, kT
            vt4 = vt4_pair[b % 2]

            # Q/K: per (which, rch) one 8-ct accumulation, split into 4
            # ~0.45us closures so the filler pacing stays fine-grained
            def qk_quads(wt, dst, bias, rch):
                st8 = {}
                def qk_u(q4):
                    def f():
                        if q4 == 0:
                            st8["acc"] = acc_ps.tile([128, 512], F32,
                                                     name="qkacc", tag="acc")
                        acc = st8["acc"]
                        for ct in range(2 * q4, 2 * q4 + 2):
                            nc.tensor.matmul(
                                acc[:], wt[:, ct * 128:(ct + 1) * 128],
                                xt[:, ct * S + rch * 512: ct * S + rch * 512 + 512],
                                start=(ct == 0), stop=(ct == 7))
                        if q4 == 3:
                            nc.vector.tensor_scalar_add(
                                dst[:, rch * 512:(rch + 1) * 512], acc[:],
                                bias[:, 0:1])
                    return f
                return [qk_u(q4) for q4 in range(4)]

            k_cls = [c for rch in range(4) for c in qk_quads(wsk, kT, bk, rch)]
            q_cls = [qk_quads(wsq, qT, bq, rch) for rch in range(4)]

            # V: per group of 4 key-tiles, 8 closures of 4-ct halves
            def v_group(g4):
                st8 = {}
                def v_u(u, h):
                    def f():
                        if u == 0 and h == 0:
                            st8["acc"] = acc_ps.tile([128, 512], F32,
                                                     name="vacc", tag="acc")
                        acc = st8["acc"]
                        kt = g4 * 4 + u
                        for ct in range(4 * h, 4 * h + 4):
                            nc.tensor.matmul(
                                acc[:, u * 128:(u + 1) * 128],
                                xt[:, ct * S + kt * 128: ct * S + (kt + 1) * 128],
                                wsv[:, ct * 128:(ct + 1) * 128],
                                start=(ct == 0), stop=(ct == CT - 1))
                        if u == 3 and h == 1:
                            nc.vector.tensor_copy(
                                vt4[:, g4 * 4:(g4 + 1) * 4, :, 0:DH],
                                acc[:].rearrange("p (u g f) -> p u g f",
                                                 u=4, f=DH))
                    return f
                return [v_u(u, h) for u in range(4) for h in range(2)]

            v_cls = [v_group(g4) for g4 in range(4)]
            return k_cls, q_cls, v_cls

        # ---- out-projection of one ich of batch b as weighted closures ----
        def make_outproj_closures(b, ich):
            OT = OTs[b]
            cls = []
            for qs in range(4 * ich, 4 * ich + 4):
                for et in range(2):
                    def y_u(b=b, qs=qs, et=et, OT=OT):
                        acc = acc_ps.tile([128, 512], F32, name="yacc", tag="acc")
                        nc.tensor.matmul(
                            acc[:], OT[:, qs * 128:(qs + 1) * 128],
                            wo[:, et * 512:(et + 1) * 512],
                            start=True, stop=True)
                        yb = ybp.tile([128, 512], BF16, name="yb")
                        nc.vector.tensor_copy(yb[:], acc[:])
                        nc.sync.dma_start(
                            ypd[b * S + qs * 128: b * S + (qs + 1) * 128,
                                et * 512:(et + 1) * 512], yb[:])
                    cls.append(y_u)
            return cls

        fillers = deque()

        def run_filler(n):
            for _ in range(n):
                if fillers:
                    fillers.popleft()()

        def drain_filler():
            while fillers:
                fillers.popleft()()

        # ---- attention for batch b ----
        def attention(b, slots=1):
            qT, kT = qTs[b], kTs[b]
            vt4 = vt4_pair[b % 2]
            OT = otp.tile([128, S], BF16)
            OTs[b] = OT
            for ich in range(NICH):
                pvs = [pv_ps.tile([128, 512], F32, tag=f"pv{h}",
                                  name=f"pv{h}") for h in range(2)]
                prev = None
                for j in range(ST):
                    sc = sc_ps.tile([128, 1024], F32)
                    for half in range(2):
                        p0 = half * 64
                        nc.tensor.matmul(
                            sc[:, half * 512:(half + 1) * 512],
                            kT[p0:p0 + 64, j * 128:(j + 1) * 128],
                            qT[p0:p0 + 64, ich * 512:(ich + 1) * 512],
                            start=True, stop=True)
                    pg = pgp.tile([128, 1024], BF16)
                    nc.scalar.activation(pg[:], sc[:], AF.Exp, scale=SCALE)
                    if prev is not None:
                        pj, ppg = prev
                        for half in range(2):
                            nc.tensor.matmul(
                                pvs[half][:], vt4[:, pj, half, :],
                                ppg[:, half * 512:(half + 1) * 512],
                                start=(pj == 0), stop=False)
                    prev = (j, pg)
                    run_filler(slots + (j % 2))
                pj, ppg = prev
                for half in range(2):
                    nc.tensor.matmul(
                        pvs[half][:], vt4[:, pj, half, :],
                        ppg[:, half * 512:(half + 1) * 512],
                        start=False, stop=True)
                # per-ich normalize: collect this ich's two denominator rows,
                # one cheap [2,512] reciprocal, broadcast, scale, bias
                dcol2 = dnp.tile([2, 512], F32, name="dcol2")
                uvs2 = []
                for half in range(2):
                    uv = uvp.tile([65, 512], F32, name="uv")
                    nc.vector.tensor_copy(uv[:], pvs[half][0:65, :])
                    nc.sync.dma_start(dcol2[half:half + 1, :], uv[64:65, :])
                    uvs2.append(uv)
                rcp2 = rcpp.tile([2, 512], F32, name="rcp2")
                nc.vector.reciprocal(rcp2[:], dcol2[:])
                for half in range(2):
                    stg = stgp.tile([1, 512], F32, name="stg")
                    nc.sync.dma_start(stg[0:1, :], rcp2[half:half + 1, :])
                    rb = rbp.tile([64, 512], F32)
                    nc.gpsimd.partition_broadcast(rb[:], stg[0:1, :])
                    o2 = o2p.tile([64, 512], F32)
                    nc.vector.tensor_mul(o2[:], uvs2[half][0:64, :], rb[:])
                    nc.vector.tensor_scalar_add(
                        OT[half * 64:half * 64 + 64,
                           ich * 512:(ich + 1) * 512],
                        o2[:], bv[half * 64:half * 64 + 64, 0:1])
                # out-projection for the PREVIOUS ich (lag keeps the PE from
                # reaching a y matmul before its OT chunk is normalized)
                if ich >= 1:
                    fillers.extend(make_outproj_closures(b, ich - 1))
            fillers.extend(make_outproj_closures(b, NICH - 1))

        # ================= emission =================
        # prefetch x of batches 0 and 1 before batch 0's filler closures can
        # reach the PE FIFO, so proj(1) never stalls the attention(0) stream
        emit_x_dma(0)
        emit_x_dma(1)
        for dst, dsrc in ((wsv, wvd), (wo, wod), (bv, bvd)):
            nc.sync.dma_start(dst[:], dsrc)
        # ~15us of dependency-free matmuls: ramps the PE clock to full rate
        # while the startup DMAs stream, so proj(0) runs warm
        for i in range(64):
            wps = acc_ps.tile([128, 512], F32, name="wps", tag="acc")
            nc.tensor.matmul(wps[:], wrm[:, 0:128], wrm[:, 0:512],
                             start=True, stop=True)
        # fast start: only K, Q(rch0) and V(group 0) inline; the rest of
        # proj(0) becomes priority filler consumed by attention(0) in an
        # order matching when attention first needs each piece
        k0, q0, v0 = make_proj_closures(0)
        for cl in k0 + q0[0] + v0[0]:
            cl()
        fillers.extend(v0[1] + v0[2] + v0[3] + q0[1] + q0[2] + q0[3])
        for b in range(B):
            if b + 1 < B:
                if b + 2 < B:
                    emit_x_dma(b + 2)
                kc, qc, vc = make_proj_closures(b + 1)
                fillers.extend(kc + [c for r in qc for c in r]
                               + [c for g in vc for c in g])
            assert len(fillers) <= (100 if b == 0 else 80), (b, len(fillers))
            attention(b, slots=2 if b == 0 else 1)
        drain_filler()

    nc.compile()
    return nc


_cache = {}


def _get_nc():
    if "nc" not in _cache:
        _cache["nc"] = build()
    return _cache["nc"]


def build_in_maps(x_q, W_qkv, b_qkv, W_out, b_out):
    x_q = np.ascontiguousarray(x_q, dtype=np.float32)
    W_qkv = np.asarray(W_qkv, dtype=np.float32)
    b_qkv = np.ascontiguousarray(b_qkv, dtype=np.float32)
    W_out = np.asarray(W_out, dtype=np.float32)
    bf = ml_dtypes.bfloat16
    # xTd[b, ct, p, s] = x[b, s, ct*128+p]
    xT = np.ascontiguousarray(
        x_q.transpose(0, 2, 1).reshape(B, CT, 128, S)).astype(bf)
    in_maps = []
    for c in range(NCORES):
        def wslice(off):
            # [p, ct*128+f] = W_qkv[off + c*128 + f, ct*128 + p]
            sl = W_qkv[off + c * 128: off + (c + 1) * 128, :]  # [f, chan]
            return np.ascontiguousarray(
                sl.reshape(128, CT, 128).transpose(2, 1, 0).reshape(
                    128, CT * 128)).astype(bf)
        wo = np.ascontiguousarray(W_out[:, c * 128:(c + 1) * 128].T).astype(bf)
        in_maps.append({
            "xT": xT,
            "wq": wslice(0),
            "wk": wslice(C),
            "wv": wslice(2 * C),
            "wo": wo,
            "bq": np.ascontiguousarray(
                b_qkv[c * 128:(c + 1) * 128].reshape(128, 1)),
            "bk": np.ascontiguousarray(
                b_qkv[C + c * 128: C + (c + 1) * 128].reshape(128, 1)),
            "bv": np.ascontiguousarray(
                b_qkv[2 * C + c * 128: 2 * C + (c + 1) * 128].reshape(128, 1)),
        })
    return in_maps


def kernel(x_q, W_qkv, b_qkv, W_out, b_out):
    """Core c computes heads (2c, 2c+1) for all batches and the partial
    out-projection against W_out[:, c*128:(c+1)*128]; the host sums the
    8 partials and adds b_out (the tensor-parallel unshard)."""
    b_out = np.ascontiguousarray(b_out, dtype=np.float32)
    nc = _get_nc()
    in_maps = build_in_maps(x_q, W_qkv, b_qkv, W_out, b_out)
    res = run_bass_kernel_spmd(nc, in_maps, list(range(NCORES)))
    y = np.zeros((B * S, C), dtype=np.float32)
    for c in range(NCORES):
        y += np.asarray(res.results[c]["yp"]).astype(np.float32)
    y += b_out[None, :]
    return y.reshape(B, S, C)


if __name__ == "__main__":
    rng = np.random.default_rng(0)
    x_q = rng.standard_normal((B, S, C), dtype=np.float32)
    s = 1.0 / np.sqrt(C)
    W_qkv = rng.uniform(-s, s, (3 * C, C)).astype(np.float32)
    b_qkv = rng.uniform(-s, s, 3 * C).astype(np.float32)
    W_out = rng.uniform(-s, s, (C, C)).astype(np.float32)
    b_out = rng.uniform(-s, s, C).astype(np.float32)
    got = kernel(x_q=x_q, W_qkv=W_qkv, b_qkv=b_qkv, W_out=W_out, b_out=b_out)
    print("smoke ok", got.shape, float(np.abs(got).max()))
